# revision 13
# baseline (speedup 1.0000x reference)
"""Trainium2 Bass kernel for nn_DefConv_386547056931 (DCNv2 block).

relu -> offset/mask conv(3x3) -> modulated deformable conv(3x3) -> BatchNorm.

Strategy (8 NeuronCores, data-parallel over batch, B=8 -> 1 image/core):
 - Offsets are small (|d|<1 for this model: w_off scaled 0.01), so bilinear
   sampling at (base+d) is EXACTLY the tent-weighted sum over integer shifts
   u,v in {-2,-1,0} around the tap: w_u(d) = max(0, 1-|d-1-u|). This removes
   all gathers: samples are weighted sums of 9 shifted reads of the
   zero-padded relu image.
 - All spatial processing uses flat PADDED coordinates in chunks of 128
   contiguous positions (pad-straddling columns are computed as garbage and
   ignored), because this toolchain requires single-free-dim matmul moving
   operands.
 - im2col is built transposed ([pixel, channel]) via PE transposes so the
   per-pixel tent*mask weights become per-partition scalars consumed by
   fused scalar_tensor_tensor MACs on the Vector engine.
 - Main conv: 9 accumulating K=128 matmuls per 256-px chunk. BatchNorm:
   valid-pixel sum/sumsq reductions, [128,2] AllReduce across the 8 cores,
   ACT scale+bias. (b_dcn cancels exactly under BN and is skipped.)
"""

import numpy as np

import bass_rust
import concourse.bass as bass
import concourse.mybir as mybir
import concourse.tile as tile
from concourse.vector_clock import ScopedClock
from concourse.bass2jax import bass_jit

# ---------------------------------------------------------------------------
# Walrus-compat patches: this toolchain rejects instructions carrying more
# than one sync-wait command; Tile freely attaches more. Split overflow waits
# onto standalone EventSemaphore instructions.
# ---------------------------------------------------------------------------
_MAX_W = 1
_orig_add_instruction = tile.TileContext._add_instruction


def _patched_add_instruction(self, inst):
    si = inst.sync_info
    if si is not None and si.on_wait is not None and len(si.on_wait) > _MAX_W:
        waits = list(si.on_wait)
        overflow, keep = waits[:-_MAX_W], waits[-_MAX_W:]
        for j in range(0, len(overflow), _MAX_W):
            w = mybir.InstEventSemaphore(name=f"{inst.name}-xw{j}")
            w.engine = inst.engine
            w.sync_info = mybir.SyncInfo(on_wait=overflow[j : j + _MAX_W], on_update=[])
            _orig_add_instruction(self, w)
        inst.sync_info = mybir.SyncInfo(on_wait=keep, on_update=list(si.on_update))
    _orig_add_instruction(self, inst)


def _patched_drain_and_barrier(self, tick_clock, wait_clock):
    vc = tick_clock.global_clock
    for proc in range(len(vc)):
        if vc[proc] > 0:
            single = bass_rust.VectorClock(
                [vc[p] if p == proc else 0 for p in range(len(vc))]
            )
            nop = self.nc.sync.nop(nofuse=True, hint="exit_wait")
            wait_clock.add_sem_waits(nop.ins, ScopedClock({None: single}))
    self.nc.sync.drain()
    self.nc.all_engine_barrier()
    popped = self.nc._tile_sem_poison_stack.pop()
    assert popped is self._sem_poison
    self.nc.clear_and_free_semaphores(list(self.sems.allocated().values()))
    self.nc.all_engine_barrier()


tile.TileContext._add_instruction = _patched_add_instruction
tile.TileContext._drain_and_barrier = _patched_drain_and_barrier

# ---------------------------------------------------------------------------
B, C, H, W = 8, 128, 80, 80
CO = 128
HW = H * W
PW, PH = W + 4, H + 4
PHW = PH * PW                      # 7056
K = 9
EPS = 1e-5
N_CORES = 8

G = 256                            # y guard for shifted chunk reads
Q0, Q1 = 128, 6912                 # padded-flat chunk range (covers valid)
NCH = (Q1 - Q0) // 128             # 53 chunks of 128 contiguous positions
VOFF = 2 * PW + 2                  # padded-flat offset of (h=0,w=0) = 170

F32 = mybir.dt.float32
MM_DT = F32
ALU = mybir.AluOpType
AF = mybir.ActivationFunctionType


def _build_device_fn():
    import jax
    from jax.sharding import Mesh, PartitionSpec
    from jax.experimental.shard_map import shard_map

    @bass_jit(factory=bass.Bass, trn_type="TRN2", num_devices=N_CORES)
    def dcn_core(nc, xb, wofft, wdcnt, boffv, gammav, betav, identv):
        out = nc.dram_tensor("out", [C, HW], F32, kind="ExternalOutput")
        om_out = nc.dram_tensor("om_out", [27, HW], F32, kind="ExternalOutput")
        sc_out = nc.dram_tensor("sc_out", [CO, 1], F32, kind="ExternalOutput")
        cc_in = nc.dram_tensor("cc_in", [CO, 2], F32)
        cc_out = nc.dram_tensor("cc_out", [CO, 2], F32, addr_space="Shared")

        with tile.TileContext(nc) as tc:
            with tc.tile_pool(name="perm", bufs=1) as perm:
                y_pad = perm.tile([C, G + PHW + G], MM_DT, name="y_pad")
                omT = perm.tile([128, NCH * 27], F32, name="omT")
                Amap = perm.tile([128, NCH * 81], F32, name="Amap")
                out_sb = perm.tile([CO, NCH * 128], F32, name="out_sb")
                woff_t = perm.tile([C, K * 27], MM_DT, name="woff_t")
                wdcn_t = perm.tile([C, K * CO], MM_DT, name="wdcn_t")
                ident = perm.tile([128, 128], MM_DT, name="ident")
                boff_t = perm.tile([27, 1], F32, name="boff_t")
                gamma_t = perm.tile([CO, 1], F32, name="gamma_t")
                beta_t = perm.tile([CO, 1], F32, name="beta_t")
                sqs_t = perm.tile([CO, 16], F32, name="sqs_t")
                stat_t = perm.tile([CO, 2], F32, name="stat_t")
                stat_r = perm.tile([CO, 2], F32, name="stat_r")
                scale_t = perm.tile([CO, 1], F32, name="scale_t")
                bias_t = perm.tile([CO, 1], F32, name="bias_t")
                scr1 = perm.tile([CO, 1], F32, name="scr1")
                scr2 = perm.tile([CO, 1], F32, name="scr2")
                eps_t = perm.tile([CO, 1], F32, name="eps_t")

                for k in range(K):
                    nc.sync.dma_start(woff_t[:, k * 27:(k + 1) * 27], wofft.ap()[k])
                    nc.sync.dma_start(wdcn_t[:, k * CO:(k + 1) * CO], wdcnt.ap()[k])
                nc.sync.dma_start(boff_t[:], boffv.ap())
                nc.sync.dma_start(gamma_t[:], gammav.ap())
                nc.sync.dma_start(beta_t[:], betav.ap())
                nc.sync.dma_start(ident[:], identv.ap())
                nc.vector.memset(eps_t[:], EPS)

                # ---- phase 1: relu+pad, offset conv, om transposes ----
                with (
                    tc.tile_pool(name="ph1", bufs=1) as ph1,
                    tc.tile_pool(name="ps1", bufs=2, space="PSUM") as ps1,
                ):
                    xt = ph1.tile([C, HW], F32, name="xt")
                    om_pad = ph1.tile([27, PHW], F32, name="om_pad")
                    nc.sync.dma_start(xt[:], xb.ap())
                    nc.gpsimd.memset(y_pad[:], 0.0)
                    y_in = y_pad[:, G:G + PHW].rearrange(
                        "c (h w) -> c h w", h=PH)[:, 2:2 + H, 2:2 + W]
                    nc.vector.tensor_scalar(
                        y_in, xt[:].rearrange("c (h w) -> c h w", h=H),
                        1.0, 0.0, ALU.mult, ALU.max)

                    # offset conv over flat padded positions [128, 6928)
                    for o0 in range(128, 6928, 400):
                        ps_om = ps1.tile([27, 400], F32, tag="om", name=f"om_{o0}")
                        for k in range(K):
                            ki, kj = k // 3, k % 3
                            koff = (ki - 1) * PW + (kj - 1)
                            nc.tensor.matmul(
                                ps_om[:], woff_t[:, k * 27:(k + 1) * 27],
                                y_pad[:, G + o0 + koff: G + o0 + koff + 400],
                                start=(k == 0), stop=(k == K - 1))
                        nc.scalar.activation(om_pad[:, o0:o0 + 400], ps_om[:],
                                             AF.Identity, bias=boff_t[:, 0:1],
                                             scale=1.0)

                    om_src = om_pad[:, VOFF:VOFF + H * PW].rearrange(
                        "c (h w) -> c h w", w=PW)[:, :, 0:W]
                    nc.sync.dma_start(
                        om_out.ap().rearrange("c (h w) -> c h w", h=H), om_src)
                    for ci in range(NCH):
                        q0 = Q0 + ci * 128
                        psT = ps1.tile([128, 27], F32, tag="psT", name=f"psT_{ci}")
                        nc.tensor.transpose(psT[:], om_pad[:27, q0:q0 + 128],
                                            ident[:27, :27])
                        nc.scalar.copy(omT[:, ci * 27:(ci + 1) * 27], psT[:])

                # ---- phase 2: tent maps ----
                def cols(off, step, cnt=9):
                    return omT[:].rearrange("p (b j) -> p b j", j=27)[
                        :, :, off:off + step * (cnt - 1) + 1:step]

                with tc.tile_pool(name="ph2", bufs=1) as ph2:
                    R_ = {}
                    for nm, off in (("y", 0), ("x", 1)):
                        d_ap = cols(off, 2)
                        r0 = ph2.tile([128, NCH * 9], F32, name=f"R{nm}0")
                        rm = ph2.tile([128, NCH * 9], F32, name=f"R{nm}m")
                        r1 = ph2.tile([128, NCH * 9], F32, name=f"R{nm}1")
                        nc.vector.tensor_scalar(r0[:], d_ap, 1.0, 0.0,
                                                ALU.mult, ALU.max)
                        nc.vector.tensor_scalar(rm[:], d_ap, -1.0, 0.0,
                                                ALU.mult, ALU.max)
                        nc.vector.tensor_tensor(r1[:], r0[:], rm[:], ALU.add)
                        nc.vector.tensor_scalar(r1[:], r1[:], -1.0, 1.0,
                                                ALU.mult, ALU.add)
                        R_[nm] = [rm, r1, r0]          # u = -2, -1, 0
                    sig = ph2.tile([128, NCH * 9], F32, name="sig")
                    nc.scalar.activation(sig[:], cols(18, 1), AF.Sigmoid)
                    MWy = []
                    for ui in range(3):
                        mw = ph2.tile([128, NCH * 9], F32, name=f"MWy{ui}")
                        nc.vector.tensor_tensor(mw[:], sig[:], R_["y"][ui][:],
                                                ALU.mult)
                        MWy.append(mw)
                    A_r = Amap[:].rearrange("p (b j) -> p b j", j=81)
                    for ui in range(3):
                        for vi in range(3):
                            o0 = (ui * 3 + vi) * 9
                            nc.vector.tensor_tensor(
                                A_r[:, :, o0:o0 + 9],
                                MWy[ui][:].rearrange("p (b j) -> p b j", j=9),
                                R_["x"][vi][:].rearrange("p (b j) -> p b j", j=9),
                                ALU.mult)

                # ---- phase 3: col build + main conv ----
                CPR = 8                                # chunks per region
                with (
                    tc.tile_pool(name="colp", bufs=1) as colp,
                    tc.tile_pool(name="ph3", bufs=3) as ph3,
                    tc.tile_pool(name="ps3", bufs=2, space="PSUM") as ps3,
                    tc.tile_pool(name="ps3y", bufs=3, space="PSUM") as ps3y,
                ):
                    RPX = CPR * 128
                    col_sb = colp.tile([C, K * RPX], MM_DT, name="col_sb")
                    for r0c in range(0, NCH, CPR):
                        ncr = min(CPR, NCH - r0c)
                        for bi in range(ncr):
                            ci = r0c + bi
                            q0 = Q0 + ci * 128
                            accs = [ph3.tile([128, C], F32, tag=f"acc{kk}",
                                             name=f"acc_{ci}_{kk}")
                                    for kk in range(K)]
                            first = [True] * K
                            for sy in range(-2, 3):
                                for sx in range(-2, 3):
                                    s = sy * PW + sx
                                    psy = ps3y.tile([128, C], MM_DT, tag="yT",
                                                    name=f"yT_{ci}_{sy}_{sx}")
                                    nc.tensor.transpose(
                                        psy[:],
                                        y_pad[:, G + q0 + s: G + q0 + s + 128],
                                        ident[:])
                                    ysb = ph3.tile([128, C], F32, tag="ysb",
                                                   name=f"ysb_{ci}_{sy}_{sx}")
                                    nc.scalar.copy(ysb[:], psy[:])
                                    for k in range(K):
                                        ki, kj = k // 3, k % 3
                                        u, v = sy - ki, sx - kj
                                        if not (-2 <= u <= 0 and -2 <= v <= 0):
                                            continue
                                        a_col = (ci * 81
                                                 + ((u + 2) * 3 + (v + 2)) * 9 + k)
                                        nc.vector.scalar_tensor_tensor(
                                            accs[k][:], ysb[:],
                                            Amap[:, a_col:a_col + 1], accs[k][:],
                                            ALU.mult,
                                            ALU.bypass if first[k] else ALU.add)
                                        first[k] = False
                            for k in range(K):
                                psc = ps3.tile([C, 128], MM_DT, tag="psc",
                                               name=f"psc_{ci}_{k}")
                                nc.tensor.transpose(psc[:], accs[k][:], ident[:])
                                nc.scalar.copy(
                                    col_sb[:, k * RPX + bi * 128:
                                           k * RPX + (bi + 1) * 128], psc[:])
                        for n0 in range(0, ncr * 128, 256):
                            nn_ = min(256, ncr * 128 - n0)
                            psm = ps3.tile([CO, 256], F32, tag="psm",
                                           name=f"psm_{r0c}_{n0}")
                            for k in range(K):
                                nc.tensor.matmul(
                                    psm[:, :nn_], wdcn_t[:, k * CO:(k + 1) * CO],
                                    col_sb[:, k * RPX + n0: k * RPX + n0 + nn_],
                                    start=(k == 0), stop=(k == K - 1))
                            gcol = r0c * 128 + n0
                            nc.scalar.copy(out_sb[:, gcol:gcol + nn_],
                                           psm[:, :nn_])

                # ---- phase 4: BN over valid pixels only ----
                # out_sb col j <-> padded-flat Q0+j; valid (h,w) at col
                # VOFF-Q0 + h*PW + w
                with tc.tile_pool(name="ph4", bufs=1) as ph4:
                    vbase = VOFF - Q0
                    val3 = out_sb[:, vbase:vbase + H * PW].rearrange(
                        "c (h w) -> c h w", w=PW)[:, :, 0:W]
                    red1 = ph4.tile([CO, H], F32, name="red1")
                    nc.vector.tensor_reduce(red1[:], val3,
                                            mybir.AxisListType.X, ALU.add)
                    nc.vector.tensor_reduce(stat_t[:, 0:1], red1[:],
                                            mybir.AxisListType.X, ALU.add)
                    sqs = ph4.tile([CO, 10 * W], F32, name="sqs")
                    for g in range(8):
                        gb = vbase + g * 10 * PW
                        gsl = out_sb[:, gb:gb + 10 * PW].rearrange(
                            "c (h w) -> c h w", w=PW)[:, :, 0:W]
                        nc.vector.scalar_tensor_tensor(
                            sqs[:].rearrange("c (h w) -> c h w", w=W), gsl, 1.0,
                            gsl, ALU.mult, ALU.mult,
                            accum_out=sqs_t[:, g:g + 1])
                    nc.vector.tensor_reduce(stat_t[:, 1:2], sqs_t[:, 0:8],
                                            mybir.AxisListType.X, ALU.add)

                cc_sem = nc.alloc_semaphore(name="cc_sem")
                dma_sem = nc.alloc_semaphore(name="dma_sem")
                with tc.tile_critical():
                    nc.sync.dma_start(cc_in.ap(), stat_t[:]).then_inc(dma_sem, 16)
                    nc.gpsimd.wait_ge(dma_sem, 16)
                    nc.gpsimd.collective_compute(
                        "AllReduce", ALU.add,
                        replica_groups=[list(range(N_CORES))],
                        ins=[cc_in.ap()], outs=[cc_out.ap()],
                    ).then_inc(cc_sem, 1)
                    nc.sync.wait_ge(cc_sem, 1)
                    nc.sync.dma_start(stat_r[:], cc_out.ap()).then_inc(dma_sem, 16)
                    nc.sync.wait_ge(dma_sem, 32)
                inv_n = 1.0 / (N_CORES * HW)
                nc.vector.tensor_scalar_mul(scr1[:], stat_r[:, 0:1], inv_n)
                nc.vector.tensor_scalar_mul(scr2[:], stat_r[:, 1:2], inv_n)
                nc.vector.scalar_tensor_tensor(
                    scr2[:], scr1[:], scr1[:, 0:1], scr2[:], ALU.mult,
                    ALU.subtract)
                nc.scalar.activation(scr2[:], scr2[:], AF.Sqrt,
                                     bias=eps_t[:, 0:1], scale=-1.0)
                nc.vector.reciprocal(scr2[:], scr2[:])
                nc.vector.tensor_tensor(scale_t[:], gamma_t[:], scr2[:], ALU.mult)
                nc.vector.scalar_tensor_tensor(
                    bias_t[:], scr1[:], scale_t[:, 0:1], beta_t[:],
                    ALU.mult, ALU.subtract)
                nc.vector.tensor_scalar_mul(bias_t[:], bias_t[:], -1.0)
                NT = NCH * 128
                for n0 in range(0, NT, 512):
                    nn_ = min(512, NT - n0)
                    nc.scalar.activation(
                        out_sb[:, n0:n0 + nn_], out_sb[:, n0:n0 + nn_],
                        AF.Identity, bias=bias_t[:, 0:1], scale=scale_t[:, 0:1])
                vbase = VOFF - Q0
                src = out_sb[:, vbase:vbase + H * PW].rearrange(
                    "c (h w) -> c h w", w=PW)[:, :, 0:W]
                dst = out.ap().rearrange("c (h w) -> c h w", h=H)
                nc.sync.dma_start(dst, src)
                nc.sync.dma_start(sc_out.ap(), scale_t[:])
        return (out, om_out, sc_out)

    devices = jax.devices()[:N_CORES]
    mesh = Mesh(np.asarray(devices), ("core",))
    in_specs = (PartitionSpec("core"),) * 7
    out_specs = (PartitionSpec("core"),) * 3
    return jax.jit(shard_map(dcn_core, mesh=mesh, in_specs=in_specs,
                             out_specs=out_specs, check_rep=False))


_FN = None


def kernel(x, w_off, b_off, w_dcn, b_dcn, gamma, beta):
    global _FN
    x = np.ascontiguousarray(np.asarray(x, dtype=np.float32))
    w_off = np.asarray(w_off, np.float32)
    w_dcn = np.asarray(w_dcn, np.float32)

    if _FN is None:
        _FN = _build_device_fn()

    wofft = np.ascontiguousarray(w_off.reshape(27, C, 9).transpose(2, 1, 0))
    wdcnt = np.ascontiguousarray(w_dcn.reshape(CO, C, 9).transpose(2, 1, 0))
    boffv = np.asarray(b_off, np.float32).reshape(27, 1)
    gammav = np.asarray(gamma, np.float32).reshape(CO, 1)
    betav = np.asarray(beta, np.float32).reshape(CO, 1)
    identv = np.eye(128, dtype=np.float32)

    xc = x.reshape(B * C, HW)
    conc = lambda a: np.concatenate([a] * N_CORES, axis=0)
    out, om, sc = _FN(xc, conc(wofft), conc(wdcnt), conc(boffv), conc(gammav),
                      conc(betav), conc(identv))
    out = np.array(out).reshape(B, C, H, W)
    om = np.asarray(om).reshape(B, 27, H, W)
    scale = np.asarray(sc).reshape(B, CO)[0]

    # Sparse exact correction for the rare |offset| > 1 pixels: the device's
    # 3-tap tent equals the true 5-tap interpolation minus the overflow
    # corner terms. Patch those pixels on the host (BN-stat shift ~1e-5).
    dyx = om[:, :18].reshape(B, 9, 2, H, W)
    ovf = np.argwhere(np.abs(dyx).max(axis=2) > 1.0)        # (b, k, h, w)
    if len(ovf):
        y = np.maximum(x, 0.0).reshape(B, C, H, W)
        us = np.arange(-3, 2, dtype=np.float32)

        def w5(d):
            return np.maximum(0.0, 1.0 - np.abs(d - 1.0 - us))

        def w3(d):
            # exactly what the device computes (unclipped middle tent)
            w = np.zeros(5, np.float32)
            w[1] = max(0.0, -d)
            w[2] = 1.0 - abs(d)
            w[3] = max(0.0, d)
            return w

        wk = w_dcn.reshape(CO, C, 9)
        for b, k, h, w_ in ovf:
            ki, kj = k // 3, k % 3
            dy = dyx[b, k, 0, h, w_]
            dx = dyx[b, k, 1, h, w_]
            m = 1.0 / (1.0 + np.exp(-om[b, 18 + k, h, w_]))
            dW = np.outer(w5(dy), w5(dx)) - np.outer(w3(dy), w3(dx))
            dcol = np.zeros(C, np.float32)
            for ui in range(5):
                for vi in range(5):
                    if dW[ui, vi] == 0.0:
                        continue
                    r, c_ = h + ki + ui - 3, w_ + kj + vi - 3
                    if 0 <= r < H and 0 <= c_ < W:
                        dcol += dW[ui, vi] * y[b, :, r, c_]
            out[b, :, h, w_] += (wk[:, :, k] @ (m * dcol)) * scale
    return out
